# revision 44
# baseline (speedup 1.0000x reference)
import os, sys
import numpy as np

sys.path.insert(0, "/opt/trn_rl_repo")

import concourse.bass as bass
import concourse.bacc as bacc
import concourse.tile as tile
import concourse.mybir as mybir
from concourse.bass_utils import run_bass_kernel_spmd

F32 = mybir.dt.float32
BF16 = mybir.dt.bfloat16
AF = mybir.ActivationFunctionType
ALU = mybir.AluOpType

NC = 8
B, C, H, W = 64, 128, 28, 28
BL = B // NC
HW = H * W
T = BL * HW                  # 6272
HEADS, D = 4, 32
E = 512
KV, L = 15, 225
EPS = 1e-5
NG = float(B * HW)
SCALE = D ** -0.5
NT, TCH = 14, 448
KC0, KC1 = 128, L - 128
NEG = -1e30

last_result = None


def _class_ranges(k):
    if k == 0:
        return (1, 2)
    if k == 1:
        return (0, 1, 2)
    return (0, 1)


def _host_prep(inputs):
    import ml_dtypes
    bf = ml_dtypes.bfloat16
    f = lambda a: np.ascontiguousarray(np.asarray(a), dtype=np.float32)
    inp = {k: np.asarray(v) for k, v in inputs.items()}
    h = {}

    def diag(wk, ntap, dt):
        ch = wk.shape[0]
        nch = ch // 128
        out = np.zeros((128, nch, ntap, 128), dtype=np.float32)
        for cc in range(nch):
            for t in range(ntap):
                out[np.arange(128), cc, t, np.arange(128)] = wk[cc * 128:(cc + 1) * 128, t]
        return np.ascontiguousarray(out.astype(dt))

    h["lpu_diag"] = diag(f(inp["lpu_w"]).reshape(C, 9), 9, bf).reshape(128, 9, 128)
    h["lpu_b"] = f(inp["lpu_b"]).reshape(C, 1)
    h["kdw_diag"] = diag(f(inp["kdw_w"]).reshape(C, 4), 4, bf).reshape(128, 4, 128)
    h["vdw_diag"] = diag(f(inp["vdw_w"]).reshape(C, 4), 4, bf).reshape(128, 4, 128)
    h["wqT"] = f(inp["wq"]).T.copy().astype(bf)
    h["wkT"] = f(inp["wk"]).T.copy().astype(bf)
    h["wvT"] = f(inp["wv"]).T.copy().astype(bf)
    h["woT"] = f(inp["wo"]).T.copy().astype(bf)
    h["bq"] = f(inp["bq"]).reshape(C, 1)
    h["bkp"] = (f(inp["bk"]) + f(inp["wk"]) @ f(inp["kdw_b"])).reshape(C, 1)
    bvp = f(inp["bv"]) + f(inp["wv"]) @ f(inp["vdw_b"])
    h["bop"] = (f(inp["bo"]) + f(inp["wo"]) @ bvp).reshape(C, 1)
    # exp(attention bias), [128 key-partitions, 2 kc, 4 heads, HW];
    # rows beyond the valid key count are 0 so the multiplied scores
    # vanish and the AV matmul can contract over the full 128 partitions
    eb = np.exp(f(inp["attn_bias"]))[0].transpose(0, 2, 1)  # [4, 225, 784]
    lb = np.zeros((128, 2, HEADS, HW), dtype=np.float32)
    lb[:, 0] = eb[:, 0:128, :].transpose(1, 0, 2)
    lb[:KC1, 1] = eb[:, 128:L, :].transpose(1, 0, 2)
    h["expb"] = np.ascontiguousarray(lb.astype(bf))
    dww = f(inp["dw_w"]).reshape(E, 3, 3).copy()
    dww[:, 1, 1] += 1.0
    h["ffn_diag"] = diag(dww.reshape(E, 9), 9, bf)
    h["dw_b"] = f(inp["dw_b"]).reshape(4, 128).T.copy()
    psum9 = np.zeros((9, 4, 128), dtype=np.float32)
    for k in range(9):
        hr, wr = _class_ranges(k // 3), _class_ranges(k % 3)
        s = dww[:, hr, :][:, :, wr].sum(axis=(1, 2))
        psum9[k] = s.reshape(4, 128)
    h["psum9"] = psum9
    ind9 = np.zeros((9, H, W), dtype=np.float32)
    hc = np.full(H, 1); hc[0] = 0; hc[-1] = 2
    wc = np.full(W, 1); wc[0] = 0; wc[-1] = 2
    for i in range(H):
        for j in range(W):
            ind9[hc[i] * 3 + wc[j], i, j] = 1.0
    h["ind9"] = ind9.reshape(9, HW).astype(bf)
    h["c1wT"] = f(inp["c1_w"]).T.copy().astype(bf)
    h["c1_b"] = f(inp["c1_b"]).reshape(4, 128).T.copy()
    # bn1_b/bn1_g in [9, 4, 128] transposed-replicated layout for the
    # border-constant math done on partitions 0..8
    btg = (f(inp["bn1_b"]) / f(inp["bn1_g"])).reshape(4, 128)
    h["btg9"] = np.ascontiguousarray(np.broadcast_to(btg, (9, 4, 128)).copy())
    h["w2T"] = f(inp["c2_w"]).T.reshape(4, 128, 128).transpose(1, 0, 2).copy()
    h["bn1_g"] = f(inp["bn1_g"]).reshape(4, 128).T.copy()
    h["bn1_b"] = f(inp["bn1_b"]).reshape(4, 128).T.copy()
    h["bnr_g"] = f(inp["bnr_g"]).reshape(4, 128).T.copy()
    h["bnr_b"] = f(inp["bnr_b"]).reshape(4, 128).T.copy()
    h["bn2_g"] = f(inp["bn2_g"]).reshape(C, 1)
    h["bn2_b"] = f(inp["bn2_b"]).reshape(C, 1)
    ln_triv = (np.allclose(inp["ln1_g"], 1) and np.allclose(inp["ln1_b"], 0)
               and np.allclose(inp["ln2_g"], 1) and np.allclose(inp["ln2_b"], 0))
    h["_ln_triv"] = ln_triv
    if not ln_triv:
        h["ln1_g"] = f(inp["ln1_g"]).reshape(1, HW)
        h["ln1_b"] = f(inp["ln1_b"]).reshape(1, HW)
        h["ln2_g"] = f(inp["ln2_g"]).reshape(1, HW)
        h["ln2_b"] = f(inp["ln2_b"]).reshape(1, HW)
    return h


def _build(ln_triv):
    nc = bacc.Bacc(None, target_bir_lowering=False, num_devices=NC)
    dt = nc.dram_tensor
    xs = dt("xs", [BL, C, H, W], F32, kind="ExternalInput")
    out_t = dt("out", [BL, C, H, W], F32, kind="ExternalOutput")
    hin = {}
    specs = [
        ("lpu_diag", [128, 9, 128], BF16), ("lpu_b", [C, 1], F32),
        ("kdw_diag", [128, 4, 128], BF16), ("vdw_diag", [128, 4, 128], BF16),
        ("wqT", [C, C], BF16), ("wkT", [C, C], BF16), ("wvT", [C, C], BF16),
        ("woT", [C, C], BF16), ("bq", [C, 1], F32), ("bkp", [C, 1], F32),
        ("bop", [C, 1], F32), ("expb", [128, 2, HEADS, HW], BF16),
        ("ffn_diag", [128, 4, 9, 128], BF16), ("dw_b", [128, 4], F32),
        ("psum9", [9, 4, 128], F32), ("ind9", [9, HW], BF16),
        ("c1wT", [C, E], BF16), ("c1_b", [128, 4], F32), ("btg9", [9, 4, 128], F32),
        ("w2T", [128, 4, 128], F32),
        ("bn1_g", [128, 4], F32), ("bn1_b", [128, 4], F32),
        ("bnr_g", [128, 4], F32), ("bnr_b", [128, 4], F32),
        ("bn2_g", [C, 1], F32), ("bn2_b", [C, 1], F32),
    ]
    if not ln_triv:
        specs += [(n, [1, HW], F32) for n in ["ln1_g", "ln1_b", "ln2_g", "ln2_b"]]
    for name, shape, d in specs:
        hin[name] = dt(name, shape, d, kind="ExternalInput")
    ar_in = {4: dt("ar4i", [1, 2], F32, kind="Internal"),
             0: dt("ar0i", [1, 2], F32, kind="Internal"),
             1: dt("ar1i", [128, 8], F32, kind="Internal"),
             2: dt("ar2i", [128, 8], F32, kind="Internal"),
             3: dt("ar3i", [128, 2], F32, kind="Internal")}
    ar_out = {4: dt("ar4o", [1, 2], F32, kind="Internal", addr_space="Shared"),
              0: dt("ar0o", [1, 2], F32, kind="Internal", addr_space="Shared"),
              1: dt("ar1o", [128, 8], F32, kind="Internal", addr_space="Shared"),
              2: dt("ar2o", [128, 8], F32, kind="Internal", addr_space="Shared"),
              3: dt("ar3o", [128, 2], F32, kind="Internal", addr_space="Shared")}
    c1_dram = dt("c1d", [128, 4], F32, kind="Internal")
    RG = [list(range(NC))]
    with tile.TileContext(nc) as tc:
        _emit(nc, tc, xs, out_t, hin, ar_in, ar_out, c1_dram, RG, ln_triv)
    if not nc.is_finalized():
        nc.finalize()
    return nc


def _emit(nc, tc, xs, out_t, hin, ar_in, ar_out, c1_dram, RG, ln_triv):
    from contextlib import ExitStack
    ctx = ExitStack()
    with ctx:
        big = ctx.enter_context(tc.tile_pool(name="big", bufs=2))
        bfp = ctx.enter_context(tc.tile_pool(name="bfp", bufs=1))
        cons = ctx.enter_context(tc.tile_pool(name="cons", bufs=1))
        small = ctx.enter_context(tc.tile_pool(name="small", bufs=1))
        ps = ctx.enter_context(tc.tile_pool(name="ps", bufs=1, space="PSUM"))

        def loadc(name):
            hh = hin[name]
            t = cons.tile(list(hh.shape), hh.dtype, tag=name, name=name)
            nc.gpsimd.dma_start(out=t, in_=hh[:])
            return t

        # tiny dummy all-reduce fired first: absorbs cross-core start
        # skew + warms the CC path while real work proceeds
        dum = small.tile([1, 2], F32, tag="dum")
        nc.vector.memset(dum, 0.0)
        nc.gpsimd.dma_start(out=ar_in[0][:], in_=dum)
        nc.gpsimd.collective_compute("AllReduce", ALU.add, RG, ins=[ar_in[0][:]], outs=[ar_out[0][:]])
        lpu_diag = loadc("lpu_diag"); lpu_b = loadc("lpu_b")
        kdw_diag = loadc("kdw_diag"); vdw_diag = loadc("vdw_diag")
        wqT = loadc("wqT"); wkT = loadc("wkT"); wvT = loadc("wvT"); woT = loadc("woT")
        bq = loadc("bq"); bkp = loadc("bkp"); bop = loadc("bop")
        # expb shares the (disjoint-lifetime) h1/h2 slot to save SBUF
        expb = bfp.tile([128, 2, HEADS, HW], BF16, tag="h1h2", name="expb")
        nc.gpsimd.dma_start(out=expb, in_=hin["expb"][:])
        ffn_diag = loadc("ffn_diag"); dw_b = loadc("dw_b")
        psum9 = loadc("psum9"); ind9t = loadc("ind9"); btg9 = loadc("btg9")
        c1wT = loadc("c1wT"); c1_b = loadc("c1_b"); w2T = loadc("w2T")
        bn1_g = loadc("bn1_g"); bn1_b = loadc("bn1_b")
        bnr_g = loadc("bnr_g"); bnr_b = loadc("bnr_b")
        bn2_g = loadc("bn2_g"); bn2_b = loadc("bn2_b")
        ind9 = ind9t.rearrange("k (h w) -> k h w", h=H)
        lns = {}
        if not ln_triv:
            for nm in ["ln1_g", "ln1_b", "ln2_g", "ln2_b"]:
                t = cons.tile([128, HW], F32, tag=nm, name=nm)
                nc.gpsimd.dma_start(out=t, in_=bass.AP(tensor=hin[nm], offset=0, ap=[[0, 128], [1, HW]]))
                lns[nm] = t
        epsT = small.tile([128, 1], F32, tag="epsT")
        nc.vector.memset(epsT, EPS)

        def pst(i):
            t = ps.tile([128, 2, 512], F32, tag=f"P{i % 4}", name="pst")
            return t

        # input load: per-sample DMAs on the sync queue (parallel with the
        # const loads on the gpsimd queue), bf16 cast on vector per sample
        xsb = big.tile([128, BL, HW], F32, tag="big")
        for b in range(BL):
            nc.sync.dma_start(out=xsb[:, b], in_=xs[b].rearrange("c h w -> c (h w)"))
        xbf = bfp.tile([128, BL, HW], BF16, tag="t1", bufs=2)
        for b in range(BL):
            nc.vector.tensor_copy(out=xbf[:, b], in_=xsb[:, b])
        xbf4 = xbf.rearrange("p b (h w) -> p b h w", h=H)

        # LPU dw (raw, bf16) + bias + residual -> x_lpu (f32)
        x_lpu = big.tile([128, BL, HW], F32, tag="big")
        xlp4 = x_lpu.rearrange("p b (h w) -> p b h w", h=H)
        xsb4 = xsb.rearrange("p b (h w) -> p b h w", h=H)

        def dw3x3(pt, dgrow, src4, base):
            first = True
            taps = [(1, 1)] + [(kh, kw) for kh in range(3) for kw in range(3) if (kh, kw) != (1, 1)]
            for n, (kh, kw) in enumerate(taps):
                r0 = max(0, 1 - kh - base); r1 = min(13, 28 - kh - base)
                c0 = max(0, 1 - kw); c1 = min(27, 28 - kw)
                if r1 < r0:
                    continue
                nc.tensor.matmul(
                    pt[:, r0:r1 + 1, c0:c1 + 1], dgrow[:, kh * 3 + kw, :],
                    src4[:, base + r0 + kh - 1: base + r1 + kh, c0 + kw - 1: c1 + kw],
                    start=first, stop=False, skip_group_check=True)
                first = False

        for b in range(BL):
            pt2 = pst(b)
            for half in range(2):
                base = 14 * half
                pth = pt2[:, half, 0:392].rearrange("p (r c) -> p r c", c=W)
                dw3x3(pth, lpu_diag, xbf4[:, b], base)
            for half in range(2):
                base = 14 * half
                pth = pt2[:, half, 0:392].rearrange("p (r c) -> p r c", c=W)
                nc.vector.scalar_tensor_tensor(
                    out=xlp4[:, b, base:base + 14, :], in0=pth, scalar=lpu_b,
                    in1=xsb4[:, b, base:base + 14, :], op0=ALU.add, op1=ALU.add)

        # LN over HW: stats on DVE, normalize on scalar (Identity with
        # per-partition scale/bias), output written as bf16
        def layer_norm(src, gname, dst_bf):
            sv = src.rearrange("p b (two q) -> p b two q", two=2)
            st = small.tile([128, BL, 2, 6], F32, tag="lnst")
            mv = small.tile([128, BL, 2], F32, tag="lnmv")
            sd = small.tile([128, BL, 1], F32, tag="lnsd")
            nm = small.tile([128, BL, 1], F32, tag="lnnm")
            for b in range(BL):
                for g2 in range(2):
                    nc.vector.bn_stats(out=st[:, b, g2], in_=sv[:, b, g2])
                nc.vector.bn_aggr(out=mv[:, b], in_=st[:, b])
                nc.scalar.activation(out=sd[:, b], in_=mv[:, b, 1:2], func=AF.Sqrt,
                                     bias=epsT, scale=1.0)
                nc.vector.reciprocal(out=sd[:, b], in_=sd[:, b])
                nc.vector.scalar_tensor_tensor(out=nm[:, b], in0=mv[:, b, 0:1], scalar=-1.0,
                                               in1=sd[:, b], op0=ALU.mult, op1=ALU.mult)
                nc.scalar.activation(out=dst_bf[:, b], in_=src[:, b], func=AF.Identity,
                                     bias=nm[:, b], scale=sd[:, b])
            if not ln_triv:
                g = lns[gname + "_g"]; bb = lns[gname + "_b"]
                for b in range(BL):
                    nc.vector.tensor_mul(out=dst_bf[:, b], in0=dst_bf[:, b], in1=g)
                    nc.vector.tensor_add(out=dst_bf[:, b], in0=dst_bf[:, b], in1=bb)

        xnbf = bfp.tile([128, BL, HW], BF16, tag="t2")
        layer_norm(x_lpu, "ln1", xnbf)
        xnbf6 = xnbf.rearrange("p b (hh t2 ww s2) -> p b hh t2 ww s2", t2=2, s2=2, hh=14)

        # Q projection -> bf16
        qbf = bfp.tile([128, BL, HW], BF16, tag="t3")
        xnbff = xnbf.rearrange("p b q -> p (b q)")
        qbff = qbf.rearrange("p b q -> p (b q)")
        for i in range(NT):
            pt = pst(i)
            nc.tensor.matmul(pt[:, 0, 0:448], wqT, xnbff[:, i * TCH:(i + 1) * TCH], start=True, stop=True)
            nc.vector.tensor_scalar(out=qbff[:, i * TCH:(i + 1) * TCH], in0=pt[:, 0, 0:448], scalar1=bq,
                                    scalar2=None, op0=ALU.add)
        # K/V strided 2x2 dw conv
        kxbf = bfp.tile([128, BL, L], BF16, tag="kxbf")
        vxbf = bfp.tile([128, BL, L], BF16, tag="vxbf")
        kx4 = kxbf.rearrange("p b (i j) -> p b i j", i=KV)
        vx4 = vxbf.rearrange("p b (i j) -> p b i j", i=KV)
        for b in range(BL):
            pt2 = pst(b)
            for ci, dg in ((0, kdw_diag), (1, vdw_diag)):
                pt = pt2[:, ci, 0:KV * KV].rearrange("p (i j) -> p i j", i=KV)
                first = True
                for kh in range(2):
                    for kw in range(2):
                        i0 = 1 - kh; j0 = 1 - kw
                        nc.tensor.matmul(
                            pt[:, i0:i0 + 14, j0:j0 + 14], dg[:, kh * 2 + kw, :],
                            xnbf6[:, b, 0:14, 1 - kh, 0:14, 1 - kw],
                            start=first, stop=False, skip_group_check=True)
                        first = False
            nc.vector.tensor_copy(out=kx4[:, b], in_=pt2[:, 0, 0:KV * KV].rearrange("p (i j) -> p i j", i=KV))
            nc.vector.tensor_copy(out=vx4[:, b], in_=pt2[:, 1, 0:KV * KV].rearrange("p (i j) -> p i j", i=KV))
        kbf = bfp.tile([128, BL, L], BF16, tag="kbf")
        kxf = kxbf.rearrange("p b l -> p (b l)")
        kbff = kbf.rearrange("p b l -> p (b l)")
        for i in range(4):
            pt = pst(i)
            nc.tensor.matmul(pt[:, 0, 0:450], wkT, kxf[:, i * 450:(i + 1) * 450], start=True, stop=True)
            nc.vector.tensor_scalar(out=kbff[:, i * 450:(i + 1) * 450], in0=pt[:, 0, 0:450], scalar1=bkp,
                                    scalar2=None, op0=ALU.add)
        # V: vaug[k, b, kc, 128] = projected v for all 4 heads
        vaug = bfp.tile([128, BL, 2, 128], BF16, tag="vaug")
        nc.vector.memset(vaug.rearrange("p b kc x -> p (b kc x)"), 0.0)
        ones32 = small.tile([128, 32], BF16, tag="ones32")
        nc.vector.memset(ones32, 1.0)
        for b in range(BL):
            for kc in range(2):
                ktM = KC0 if kc == 0 else KC1
                pt = pst(kc)
                nc.tensor.matmul(pt[0:ktM, 0, 0:128], vxbf[:, b, kc * 128: kc * 128 + ktM], wvT,
                                 start=True, stop=True)
                nc.vector.tensor_copy(out=vaug[0:ktM, b, kc], in_=pt[0:ktM, 0, 0:128])

        # ---------------- attention ----------------
        # QK/exp/mul waves for sample b are emitted BEFORE the AV of
        # sample b-1, so the scalar exp stream never stalls behind AV
        # matmuls in the in-order PE queue.
        o_bf = bfp.tile([128, BL, HW], BF16, tag="t2")

        def emit_av(b, et):
            pto = pst(2)
            ptd = pst(3)
            for qc in range(2):
                for kc in range(2):
                    for hd in range(HEADS):
                        nc.tensor.matmul(
                            pto[hd * 32:(hd + 1) * 32, qc, 0:392],
                            vaug[:, b, kc, hd * 32:(hd + 1) * 32],
                            et[:, kc, hd, qc * 392:(qc + 1) * 392],
                            start=(kc == 0), stop=(kc == 1),
                            tile_position=(0, hd * 32), skip_group_check=True)
                    for hd in range(HEADS):
                        nc.tensor.matmul(
                            ptd[hd * 32:(hd + 1) * 32, qc, 0:392],
                            ones32, et[:, kc, hd, qc * 392:(qc + 1) * 392],
                            start=(kc == 0), stop=(kc == 1),
                            tile_position=(0, hd * 32), skip_group_check=True)
            rf = small.tile([128, 2, 392], F32, tag="rf")
            nc.vector.reciprocal_approx_fast(out=rf, in_=ptd[:, :, 0:392])
            nc.vector.tensor_mul(
                out=o_bf[:, b].rearrange("p (a q) -> p a q", a=2),
                in0=pto[:, :, 0:392], in1=rf)

        prev = None
        for b in range(BL):
            et = bfp.tile([128, 2, HEADS, HW], BF16, tag="t1", bufs=2)
            if b < 2:
                # zero the invalid-key rows of the kc=1 half once per buffer;
                # later samples only ever write rows [0:ktM]
                nc.vector.memset(et[96:128, 1].rearrange("p h q -> p (h q)"), 0.0)
            widx = 0
            for kc in range(2):
                ktM = KC0 if kc == 0 else KC1
                for qc in range(2):
                    for pr in range(2):
                        pt2 = pst(widx % 2)
                        widx += 1
                        for j in range(2):
                            hd = pr * 2 + j
                            nc.tensor.matmul(
                                pt2[0:ktM, j, 0:392],
                                kbf[hd * 32:(hd + 1) * 32, b, kc * 128: kc * 128 + ktM],
                                qbf[hd * 32:(hd + 1) * 32, b, qc * 392:(qc + 1) * 392],
                                start=True, stop=True, skip_group_check=True,
                                tile_position=(hd * 32, 0))
                        nc.scalar.activation(
                            out=et[0:ktM, kc, 2 * pr:2 * pr + 2, qc * 392:(qc + 1) * 392],
                            in_=pt2[0:ktM, :, 0:392], func=AF.Exp, scale=SCALE)
                        nc.vector.tensor_mul(
                            out=et[0:ktM, kc, 2 * pr:2 * pr + 2, qc * 392:(qc + 1) * 392],
                            in0=et[0:ktM, kc, 2 * pr:2 * pr + 2, qc * 392:(qc + 1) * 392],
                            in1=expb[0:ktM, kc, 2 * pr:2 * pr + 2, qc * 392:(qc + 1) * 392])
            if prev is not None:
                emit_av(*prev)
            prev = (b, et)
        emit_av(*prev)

        # O projection + residual -> x_mhsa (f32)
        x_mhsa = big.tile([128, BL, HW], F32, tag="big")
        of = o_bf.rearrange("p b q -> p (b q)")
        xmf = x_mhsa.rearrange("p b q -> p (b q)")
        xlf = x_lpu.rearrange("p b q -> p (b q)")
        for i in range(NT):
            pt = pst(i)
            nc.tensor.matmul(pt[:, 0, 0:448], woT, of[:, i * TCH:(i + 1) * TCH], start=True, stop=True)
            nc.vector.scalar_tensor_tensor(out=xmf[:, i * TCH:(i + 1) * TCH], in0=pt[:, 0, 0:448],
                                           scalar=bop, in1=xlf[:, i * TCH:(i + 1) * TCH],
                                           op0=ALU.add, op1=ALU.add)

        ybf = bfp.tile([128, BL, HW], BF16, tag="t3")
        layer_norm(x_mhsa, "ln2", ybf)

        def bn_pack_reduce_mv(mv, nchunk, ar_i, ar_o):
            stats = small.tile([128, nchunk, 2], F32, tag="bnpack")
            m2s = small.tile([128, nchunk], F32, tag="bnm2s")
            nc.vector.tensor_scalar(out=stats[:, :, 0:1], in0=mv[:, :, 0:1], scalar1=float(T),
                                    scalar2=None, op0=ALU.mult)
            nc.vector.tensor_mul(out=m2s, in0=mv[:, :, 0], in1=mv[:, :, 0])
            nc.vector.tensor_add(out=m2s, in0=m2s, in1=mv[:, :, 1])
            nc.vector.tensor_scalar(out=stats[:, :, 1:2], in0=m2s.rearrange("p (e o) -> p e o", o=1),
                                    scalar1=float(T), scalar2=None, op0=ALU.mult)
            nc.gpsimd.dma_start(out=ar_i[:], in_=stats.rearrange("p e two -> p (e two)"))
            nc.gpsimd.collective_compute("AllReduce", ALU.add, RG, ins=[ar_i[:]], outs=[ar_o[:]])
            g = small.tile([128, nchunk, 2], F32, tag="bngl")
            nc.gpsimd.dma_start(out=g.rearrange("p e two -> p (e two)"), in_=ar_o[:])
            return g

        def bn_affine(gs, nchunk, gt, bt):
            # gs holds [sum(x), sum(x^2)]
            a = small.tile([128, nchunk], F32, tag="bna", bufs=3)
            cc = small.tile([128, nchunk], F32, tag="bnc", bufs=3)
            mean = small.tile([128, nchunk], F32, tag="bnmean")
            m2 = small.tile([128, nchunk], F32, tag="bnm2b")
            nc.vector.tensor_scalar(out=mean, in0=gs[:, :, 0], scalar1=1.0 / NG, scalar2=None, op0=ALU.mult)
            nc.vector.tensor_scalar(out=a, in0=gs[:, :, 1], scalar1=1.0 / NG, scalar2=None, op0=ALU.mult)
            nc.vector.tensor_mul(out=m2, in0=mean, in1=mean)
            nc.vector.tensor_sub(out=a, in0=a, in1=m2)
            nc.scalar.activation(out=a, in_=a, func=AF.Sqrt, bias=epsT, scale=1.0)
            nc.vector.reciprocal(out=a, in_=a)
            nc.vector.tensor_mul(out=a, in0=a, in1=gt)
            nc.vector.tensor_mul(out=cc, in0=mean, in1=a)
            nc.vector.scalar_tensor_tensor(out=cc, in0=cc, scalar=-1.0, in1=bt,
                                           op0=ALU.mult, op1=ALU.add)
            return a, cc

        # pw1 + gelu -> h1bf; sums via scalar accum_out, sumsq via fused
        # square-reduce on DVE; all-reduce fires right after last chunk
        h1bf = bfp.tile([128, 4, BL, HW], BF16, tag="h1h2")
        h1f = h1bf.rearrange("p e b q -> p e (b q)")
        h1r = h1f.rearrange("p e (n q) -> p e n q", q=TCH)
        ybff = ybf.rearrange("p b q -> p (b q)")
        st1 = small.tile([128, 4, NT, 6], F32, tag="sums1")
        mv1 = small.tile([128, 4, 2], F32, tag="sqs1")
        scr = small.tile([128, 784], F32, tag="rf")
        for ec in range(4):
            for i in range(NT):
                pt = pst(i)
                nc.tensor.matmul(pt[:, 0, 0:448], c1wT[:, ec * 128:(ec + 1) * 128],
                                 ybff[:, i * TCH:(i + 1) * TCH], start=True, stop=True)
                nc.scalar.activation(out=h1f[:, ec, i * TCH:(i + 1) * TCH], in_=pt[:, 0, 0:448],
                                     func=AF.Gelu, bias=c1_b[:, ec:ec + 1], scale=1.0)
            for i in range(NT):
                nc.vector.bn_stats(out=st1[:, ec, i], in_=h1r[:, ec, i])
            nc.vector.bn_aggr(out=mv1[:, ec], in_=st1[:, ec])
        gs1 = bn_pack_reduce_mv(mv1, 4, ar_in[1], ar_out[1])
        a1, c1 = bn_affine(gs1, 4, bn1_g, bn1_b)

        # border constants via transposed stats on partitions 0..8:
        # craT = (bn1_b/bn1_g)*sigma - mean ; lh9 = psum9 * craT
        gsT = small.tile([1, 4, 2, 128], F32, tag="gsT")
        nc.gpsimd.dma_start(out=gsT, in_=bass.AP(tensor=ar_out[1], offset=0,
                                                 ap=[[0, 1], [2, 4], [1, 2], [8, 128]]))
        gsT9 = small.tile([9, 4, 2, 128], F32, tag="gsT9")
        nc.gpsimd.partition_broadcast(gsT9, gsT)
        meanT = small.tile([9, 4, 128], F32, tag="meanT")
        varT = small.tile([9, 4, 128], F32, tag="varT")
        nc.vector.tensor_scalar(out=meanT, in0=gsT9[:, :, 0], scalar1=1.0 / NG, scalar2=None, op0=ALU.mult)
        nc.vector.tensor_scalar(out=varT, in0=gsT9[:, :, 1], scalar1=1.0 / NG, scalar2=None, op0=ALU.mult)
        msq = small.tile([9, 4, 128], F32, tag="gsT")
        nc.vector.tensor_mul(out=msq, in0=meanT, in1=meanT)
        nc.vector.tensor_sub(out=varT, in0=varT, in1=msq)
        nc.scalar.activation(out=varT, in_=varT, func=AF.Sqrt, bias=epsT[0:9], scale=1.0)
        craT = small.tile([9, 4, 128], F32, tag="gsT9")
        nc.vector.tensor_mul(out=craT, in0=btg9, in1=varT)
        nc.vector.tensor_sub(out=craT, in0=craT, in1=meanT)
        lh9 = small.tile([9, 4, 128], BF16, tag="lh9")
        nc.vector.tensor_mul(out=lh9, in0=psum9, in1=craT)

        # FFN dw (raw taps; BN1 scale applied at eviction) + gelu -> h2g
        # software-pipelined: raw taps run DEPTH groups ahead of the
        # border matmul + eviction (which wait on the BN1 all-reduce)
        h2g = h1bf
        h2g4 = h2g.rearrange("p e b (h w) -> p e b h w", h=H)
        h1b4 = h1bf.rearrange("p e b (h w) -> p e b h w", h=H)
        h2f = h2g.rearrange("p e b q -> p e (b q)")
        st2 = small.tile([128, 4, BL, 2, 6], F32, tag="sums2")
        mv2 = small.tile([128, 4, 2], F32, tag="sqs2")
        groups = [(ec, b) for ec in range(4) for b in range(BL)]
        pts = {}

        def conv_front(gidx):
            ec, b = groups[gidx]
            pt2 = pst(gidx)
            pts[gidx] = pt2
            for half in range(2):
                base = 14 * half
                pth = pt2[:, half, 0:392].rearrange("p (r c) -> p r c", c=W)
                dw3x3(pth, ffn_diag[:, ec], h1b4[:, ec, b], base)

        def conv_back(gidx):
            ec, b = groups[gidx]
            pt2 = pts.pop(gidx)
            for half in range(2):
                base = 14 * half
                pth = pt2[:, half, 0:392].rearrange("p (r c) -> p r c", c=W)
                nc.tensor.matmul(pth, lh9[:, ec], ind9[:, base:base + 14, :],
                                 start=False, stop=True, skip_group_check=True)
            for half in range(2):
                base = 14 * half
                pth = pt2[:, half, 0:392].rearrange("p (r c) -> p r c", c=W)
                nc.scalar.activation(out=h2g4[:, ec, b, base:base + 14, :], in_=pth,
                                     func=AF.Gelu, bias=dw_b[:, ec:ec + 1],
                                     scale=a1[:, ec:ec + 1])
            for half in range(2):
                nc.vector.bn_stats(out=st2[:, ec, b, half],
                                   in_=h2f[:, ec, b * HW + half * 392: b * HW + (half + 1) * 392])

        # groups 0..NDEF-1 (all of ec=0) are raw-evicted to SBUF so the
        # PE keeps streaming taps while the BN1 all-reduce is in flight;
        # their border+gelu are applied afterwards from SBUF.
        NDEF = 8
        h2tmp = bfp.tile([128, BL, HW], BF16, tag="t3", name="h2tmp")
        border_t = small.tile([128, 2, 392], BF16, tag="rf", name="border_t")

        def raw_evict(gidx):
            ec, b = groups[gidx]
            pt2 = pts.pop(gidx)
            for half in range(2):
                nc.scalar.copy(out=h2tmp[:, b, half * 392:(half + 1) * 392],
                               in_=pt2[:, half, 0:392])

        def deferred_back(gidx):
            ec, b = groups[gidx]  # ec == 0
            for half in range(2):
                nc.vector.tensor_add(
                    out=h2tmp[:, b, half * 392:(half + 1) * 392],
                    in0=h2tmp[:, b, half * 392:(half + 1) * 392],
                    in1=border_t[:, half])
            for half in range(2):
                base = 14 * half
                nc.scalar.activation(
                    out=h2g4[:, 0, b, base:base + 14, :],
                    in_=h2tmp[:, b, half * 392:(half + 1) * 392].rearrange("p (r c) -> p r c", c=W),
                    func=AF.Gelu, bias=dw_b[:, 0:1], scale=a1[:, 0:1])
            for half in range(2):
                nc.vector.bn_stats(out=st2[:, 0, b, half],
                                   in_=h2f[:, 0, b * HW + half * 392: b * HW + (half + 1) * 392])

        for g in range(NDEF):
            conv_front(g)
            raw_evict(g)
        # border constants for ec=0 (stalls PE on the all-reduce, with
        # NDEF groups of raw taps already buffered ahead of it)
        bt = pst(NDEF)
        for half in range(2):
            pth = bt[:, half, 0:392].rearrange("p (r c) -> p r c", c=W)
            nc.tensor.matmul(pth, lh9[:, 0], ind9[:, 14 * half:14 * half + 14, :],
                             start=True, stop=True, skip_group_check=True)
        for half in range(2):
            nc.scalar.copy(out=border_t[:, half], in_=bt[:, half, 0:392])
        for g in range(NDEF, NDEF + 4):
            conv_front(g)
        for g in range(NDEF):
            deferred_back(g)
        for g in range(NDEF, len(groups)):
            conv_back(g)
            if g + 4 < len(groups):
                conv_front(g + 4)
        for ec in range(4):
            nc.vector.bn_aggr(out=mv2[:, ec], in_=st2[:, ec].rearrange("p b h s -> p (b h) s"))
        gs2 = bn_pack_reduce_mv(mv2, 4, ar_in[2], ar_out[2])
        a2, c2 = bn_affine(gs2, 4, bnr_g, bnr_b)
        w2s = bfp.tile([128, 4, 128], BF16, tag="w2s")
        for kc in range(4):
            nc.vector.tensor_scalar(out=w2s[:, kc], in0=w2T[:, kc], scalar1=a2[:, kc:kc + 1],
                                    scalar2=None, op0=ALU.mult)
        ptb = pst(0)
        for kc in range(4):
            nc.tensor.matmul(ptb[:, 0, 0:1], w2T[:, kc], c2[:, kc:kc + 1], start=(kc == 0), stop=(kc == 3))
        biasc = small.tile([128, 1], F32, tag="biascS")
        nc.vector.tensor_copy(out=biasc, in_=ptb[:, 0, 0:1])

        # pw2 -> h3s (scalar eviction w/ sum accum; DVE fused sq-reduce)
        h3s = big.tile([128, BL, HW], F32, tag="big")
        h3f = h3s.rearrange("p b q -> p (b q)")
        h3r = h3f.rearrange("p (n q) -> p n q", q=TCH)
        st3 = small.tile([128, NT, 6], F32, tag="sums3")
        mv3 = small.tile([128, 1, 2], F32, tag="sqs3")
        for i in range(NT):
            pt = pst(i)
            for kc in range(4):
                nc.tensor.matmul(pt[:, 0, 0:448], w2s[:, kc], h2f[:, kc, i * TCH:(i + 1) * TCH],
                                 start=(kc == 0), stop=(kc == 3))
            nc.scalar.activation(out=h3f[:, i * TCH:(i + 1) * TCH], in_=pt[:, 0, 0:448],
                                 func=AF.Identity, bias=biasc, scale=1.0)
            nc.vector.bn_stats(out=st3[:, i], in_=h3r[:, i])
        nc.vector.bn_aggr(out=mv3[:, 0], in_=st3)
        gs3 = bn_pack_reduce_mv(mv3, 1, ar_in[3], ar_out[3])
        a3, c3 = bn_affine(gs3, 1, bn2_g, bn2_b)

        for b in range(BL):
            nc.scalar.activation(out=h3s[:, b], in_=h3s[:, b], func=AF.Identity,
                                 bias=c3, scale=a3)
            nc.vector.tensor_add(out=x_mhsa[:, b], in0=x_mhsa[:, b], in1=h3s[:, b])
            nc.sync.dma_start(out=out_t[b].rearrange("c h w -> c (h w)"), in_=x_mhsa[:, b])


_cached = None


def kernel(**inputs):
    global last_result, _cached
    hp = _host_prep(inputs)
    ln_triv = hp.pop("_ln_triv")
    if _cached is None or _cached[1] != ln_triv:
        _cached = (_build(ln_triv), ln_triv)
    nc = _cached[0]
    x = np.ascontiguousarray(np.asarray(inputs["x"], dtype=np.float32))
    in_maps = []
    for c in range(NC):
        m = dict(hp)
        m["xs"] = np.ascontiguousarray(x[c * BL:(c + 1) * BL])
        in_maps.append(m)
    trace = os.environ.get("KERNEL_TRACE", "0") == "1"
    res = run_bass_kernel_spmd(nc, in_maps, core_ids=list(range(NC)), trace=trace)
    last_result = res
    return np.concatenate([r["out"] for r in res.results], axis=0)
